# revision 3
# baseline (speedup 1.0000x reference)
"""Segment mean-pooling (sorted segment ids) on 8 TRN2 NeuronCores.

Strategy
--------
Batch rows (B=8) are sharded one-per-core (data parallel, no collectives).
Per core: segment-sum of feats [S=8192, H=512] into G=1024 groups is computed
as a banded one-hot matmul on the PE:

    out[g, h] = sum_t onehot[t, g] * feats[t, h]

with tokens tiled 128-per-matmul (the PE contraction dim).  Because ids are
sorted, each 128-token tile only touches 1-2 tiles of 128 groups; the band
(which group-tiles each token-tile feeds) is computed on the host from the
actual segment_ids and baked into the unrolled program (all-zero one-hot
blocks are harmless, so the band only needs to be a superset of the support;
we build it as the exact union over the 8 batch rows).

One-hot blocks are built on the fly by the vector engine with a single fused
tensor_scalar:  onehot[t, g] = is_equal(iota[g] - ids[t], -128*m).

float32r matmul inputs stream at 1 cycle/row (vs 4 for fp32), keeping the
kernel memory-bound (~16 MiB feats in + 2 MiB out per core).  Counts come
from a second tiny matmul (N=1, rhs=ones) into a separate PSUM bank; means
are produced by DVE reciprocal + per-partition scale.
"""

import numpy as np

P = 128          # partitions / tokens per matmul
S = 8192         # tokens per batch row
H = 512          # feature dim
G = 1024         # groups per batch row
B = 8            # batch rows == cores
KT = S // P      # 64 token tiles
MT = G // P      # 8 group tiles
TILES_PER_DMA = 4  # 4 token tiles => 1 MiB per feats DMA

_CACHE: dict[tuple, object] = {}
LAST_RESULTS = None  # BassKernelResults of the most recent run (for test.py)
TRACE = False        # set by test.py to capture an NTFF profile


def _compute_band(segment_ids: np.ndarray) -> tuple[tuple[int, ...], tuple[int, ...]]:
    """For each group-tile m, the union over batch rows of [first, last]
    token-tile containing a token of groups [128m, 128m+128)."""
    kf = [MT * m for m in range(MT)]  # defaults for empty tiles (any value ok)
    kl = [MT * m for m in range(MT)]
    for b in range(B):
        ids = segment_ids[b]
        mt = ids // P                       # group-tile of each token
        for m in range(MT):
            pos = np.nonzero(mt == m)[0]
            if pos.size:
                kf[m] = min(kf[m], int(pos[0]) // P)
                kl[m] = max(kl[m], int(pos[-1]) // P)
    # monotone clip (sortedness guarantees this already; cheap safety)
    for m in range(1, MT):
        kf[m] = max(kf[m], kf[m - 1])
        kl[m] = max(kl[m], kl[m - 1])
    return tuple(kf), tuple(kl)


def _build_program(kf, kl):
    import concourse.bacc as bacc
    import concourse.mybir as mybir
    import concourse.tile as tile

    f32 = mybir.dt.float32
    f32r = mybir.dt.float32r
    i32 = mybir.dt.int32
    sub = mybir.AluOpType.subtract
    iseq = mybir.AluOpType.is_equal

    # which group-tiles each token-tile feeds
    m_of_k = [[m for m in range(MT) if kf[m] <= k <= kl[m]] for k in range(KT)]
    max_live = max(len(ms) for ms in m_of_k)
    psum_bufs = min(4, max_live + 1)

    nc = bacc.Bacc("TRN2", target_bir_lowering=False, debug=False, num_devices=B)

    FE = nc.dram_tensor("FE", [S, H], f32r, kind="ExternalInput")
    IDS = nc.dram_tensor("IDS", [P, KT], f32, kind="ExternalInput")
    IOTA = nc.dram_tensor("IOTA", [P, P], f32, kind="ExternalInput")
    ONES = nc.dram_tensor("ONES", [P, 2], f32r, kind="ExternalInput")
    GR = nc.dram_tensor("GR", [G, H], f32, kind="ExternalOutput")
    CN = nc.dram_tensor("CN", [P, MT], i32, kind="ExternalOutput")

    n_dma = KT // TILES_PER_DMA
    fe_src = FE.ap().rearrange("(d j p) h -> d p j h", p=P, j=TILES_PER_DMA)

    with tile.TileContext(nc) as tc:
        with (
            tc.tile_pool(name="const", bufs=1) as const,
            tc.tile_pool(name="feats", bufs=4) as featsp,
            tc.tile_pool(name="oh", bufs=6) as ohp,
            tc.tile_pool(name="outp", bufs=3) as outp,
            tc.tile_pool(name="small", bufs=4) as small,
            tc.tile_pool(name="pm", bufs=psum_bufs, space="PSUM") as pmp,
            tc.tile_pool(name="pc", bufs=psum_bufs, space="PSUM") as pcp,
        ):
            iota = const.tile([P, P], f32)
            ids = const.tile([P, KT], f32)
            ones = const.tile([P, 2], f32r)
            cnt_i32 = const.tile([P, MT], i32)
            nc.sync.dma_start(out=iota[:], in_=IOTA.ap())
            nc.sync.dma_start(out=ids[:], in_=IDS.ap())
            nc.sync.dma_start(out=ones[:], in_=ONES.ap())

            psum_m = {}
            psum_c = {}
            ftile = None
            for k in range(KT):
                j = k % TILES_PER_DMA
                if j == 0:
                    d = k // TILES_PER_DMA
                    ftile = featsp.tile([P, TILES_PER_DMA, H], f32r, tag="ft")
                    nc.sync.dma_start(out=ftile[:], in_=fe_src[d])
                fk = ftile[:, j, :]
                for m in m_of_k[k]:
                    oh = ohp.tile([P, P], f32r, tag="oh")
                    nc.vector.tensor_scalar(
                        out=oh[:], in0=iota[:],
                        scalar1=ids[:, k:k + 1], scalar2=float(-P * m),
                        op0=sub, op1=iseq,
                    )
                    if k == kf[m]:
                        psum_m[m] = pmp.tile([P, H], f32, tag="pm", name=f"pm{m}")
                        psum_c[m] = pcp.tile([P, 2], f32, tag="pc", name=f"pc{m}")
                    nc.tensor.matmul(
                        out=psum_m[m][:], lhsT=oh[:], rhs=fk,
                        start=(k == kf[m]), stop=(k == kl[m]),
                    )
                    nc.tensor.matmul(
                        out=psum_c[m][:], lhsT=oh[:], rhs=ones[:],
                        start=(k == kf[m]), stop=(k == kl[m]),
                    )
                    if k == kl[m]:
                        cmax = small.tile([P, 1], f32, tag="cmax")
                        rec = small.tile([P, 1], f32, tag="rec")
                        nc.vector.tensor_scalar_max(
                            out=cmax[:], in0=psum_c[m][:, 0:1], scalar1=1.0)
                        nc.vector.reciprocal(out=rec[:], in_=cmax[:])
                        om = outp.tile([P, H], f32, tag="om")
                        nc.vector.tensor_scalar(
                            out=om[:], in0=psum_m[m][:],
                            scalar1=rec[:, 0:1], scalar2=None,
                            op0=mybir.AluOpType.mult,
                        )
                        nc.sync.dma_start(
                            out=GR.ap()[m * P:(m + 1) * P, :], in_=om[:])
                        nc.vector.tensor_copy(
                            out=cnt_i32[:, m:m + 1], in_=psum_c[m][:, 0:1])
            nc.sync.dma_start(out=CN.ap(), in_=cnt_i32[:])

    nc.compile()
    return nc


def kernel(feats: np.ndarray, segment_ids: np.ndarray):
    global LAST_RESULTS
    from concourse.bass_utils import run_bass_kernel_spmd

    assert feats.shape == (B, S, H) and segment_ids.shape == (B, S)
    feats = np.ascontiguousarray(feats, dtype=np.float32)
    segment_ids = np.ascontiguousarray(segment_ids, dtype=np.int32)

    kf, kl = _compute_band(segment_ids)
    key = (kf, kl)
    if key not in _CACHE:
        _CACHE[key] = _build_program(kf, kl)
    nc = _CACHE[key]

    iota_np = np.tile(np.arange(P, dtype=np.float32), (P, 1))
    ones_np = np.ones((P, 2), dtype=np.float32)
    in_maps = []
    for b in range(B):
        ids_f32 = segment_ids[b].reshape(KT, P).T.astype(np.float32)
        in_maps.append({
            "FE": feats[b],
            "IDS": np.ascontiguousarray(ids_f32),
            "IOTA": iota_np,
            "ONES": ones_np,
        })

    res = run_bass_kernel_spmd(nc, in_maps, core_ids=list(range(B)), trace=TRACE)
    LAST_RESULTS = res

    grouped = np.stack([res.results[b]["GR"] for b in range(B)])
    counts = np.stack([
        res.results[b]["CN"].T.reshape(G) for b in range(B)
    ]).astype(np.int32)
    return grouped, counts
